# revision 12
# baseline (speedup 1.0000x reference)
"""Trainium2 Bass kernel for nn_MinEuclideanDistBlock (v3).

Math (reference):
  x: (B=64, C=3, L=2048), shapelets: (C=3, N=256, S=64)
  W = L - S + 1 = 1985 sliding windows
  d2[b,c,w,n] = |win|^2 + |shp|^2 - 2 win.shp    (win = x[b,c,w:w+S])
  d = sqrt(max(d2, 0));  out[b,0,n] = min_w sum_c d[b,c,w,n]

Device strategy (per core, batch-sharded B/8 = 8 batches per core):
  - T tile [128, W] per (b, c): rows 0..63  = x[b,c,w+s]  (overlap-AP DMA)
                                rows 64..127 = x^2[b,c,w+s] (overlap-AP DMA)
  - lhsT [128, 128] per (c, nt): rows 0..63 = -2*shp^T, rows 64..127 = 1.0
    One K=128 bf16 matmul chunk gives psum[n, w] = -2*cross + win_sq
    (win_sq folded into the contraction via the x^2 rows -> no serial
    moving-sum prep phase at all).
  - ACT: d = sqrt(psum + bias), bias = shp_sq[n] per partition; bf16 out.
  - DVE: acc = d0 + d1 (2x bf16), then fused tensor_tensor_reduce
    (acc + d2 -> min over w) in one pass.
"""

import numpy as np

S = 64
NSH = 256
C = 3
B = 64
L = 2048
W = L - S + 1  # 1985
NCORES = 8
BPC = B // NCORES  # 8
NT = 2  # shapelet tiles of 128
WCHUNKS = [(0, 512), (512, 512), (1024, 512), (1536, W - 1536)]
# tile indices (0..15) whose c1-add / c2-add run on GPSIMD instead of DVE
GPS1 = set()
GPS2 = set()

_cache = {}


def _build_nc(reps=1, ablate=()):
    import concourse.bass as bass
    import concourse.bacc as bacc
    import concourse.mybir as mybir
    import concourse.tile as tile

    f32 = mybir.dt.float32
    bf16 = mybir.dt.bfloat16

    nc = bacc.Bacc()
    xs = nc.dram_tensor("xs", [BPC, C, L], bf16, kind="ExternalInput")
    xq = nc.dram_tensor("xq", [BPC, C, L], bf16, kind="ExternalInput")
    wts = nc.dram_tensor("wts", [C, NT, 128, 128], bf16, kind="ExternalInput")
    ssq = nc.dram_tensor("ssq", [C, NT, 128], f32, kind="ExternalInput")
    out = nc.dram_tensor("out", [BPC, NT, 128], f32, kind="ExternalOutput")

    deep = 2 if "bufs6" in ablate else 0
    with tile.TileContext(nc) as tc:
        with (
            tc.tile_pool(name="consts", bufs=1) as consts,
            tc.tile_pool(name="tpool", bufs=4 + deep) as tpool,
            tc.tile_pool(name="psump", bufs=2, space="PSUM") as psump,
            tc.tile_pool(name="accp", bufs=4 + deep) as accp,
            tc.tile_pool(name="tmpp", bufs=4 + deep) as tmpp,
            tc.tile_pool(name="scrp", bufs=6 + deep) as scrp,
            tc.tile_pool(name="minvp", bufs=8) as minvp,
        ):
            # ---- constants ----
            w_all = consts.tile([128, C * NT * 128], bf16)
            biases = {}
            for c in range(C):
                for nt in range(NT):
                    idx = c * NT + nt
                    nc.sync.dma_start(
                        out=w_all[:, idx * 128 : (idx + 1) * 128],
                        in_=wts[c, nt, :, :],
                    )
                    bt = consts.tile([128, 1], f32, name=f"bias_{c}_{nt}")
                    nc.sync.dma_start(out=bt, in_=ssq[c, nt, :])
                    biases[(c, nt)] = bt

            for _rep in range(reps):
                _body(nc, tc, bass, mybir, tpool, psump, accp, tmpp, scrp,
                      minvp, xs, xq, out, w_all, biases, ablate)
    nc.compile()
    return nc


def _body(nc, tc, bass, mybir, tpool, psump, accp, tmpp, scrp, minvp,
          xs, xq, out, w_all, biases, ablate=()):
    f32 = mybir.dt.float32
    bf16 = mybir.dt.bfloat16
    AF = mybir.ActivationFunctionType
    ALU = mybir.AluOpType

    accs = {}
    for b in range(BPC):
        for c in range(C):
            T = tpool.tile([128, L], bf16, name="T")
            base = xs[b, c, :]
            apov = bass.AP(
                tensor=base.tensor,
                offset=base.offset,
                ap=[[1, S], [1, W]],
            )
            nc.sync.dma_start(out=T[0:S, 0:W], in_=apov)
            baseq = xq[b, c, :]
            apovq = bass.AP(
                tensor=baseq.tensor,
                offset=baseq.offset,
                ap=[[1, S], [1, W]],
            )
            nc.sync.dma_start(out=T[S : 2 * S, 0:W], in_=apovq)

            for nt in range(NT):
                idx = c * NT + nt
                lhsT = w_all[:, idx * 128 : (idx + 1) * 128]
                ps = psump.tile([128, 2048], f32, name="ps")
                if "nomm" not in ablate:
                    for w0, wl in WCHUNKS:
                        nc.tensor.matmul(
                            ps[:, w0 : w0 + wl],
                            lhsT=lhsT,
                            rhs=T[:, w0 : w0 + wl],
                            start=True,
                            stop=True,
                        )
                AW = 992 if "halfw" in ablate else W
                if "actmin" in ablate:
                    AW = 8
                bias_kw = (
                    {} if "nobias" in ablate
                    else {"bias": biases[(c, nt)]}
                )
                if "dvemin" in ablate:
                    # tiny DVE consumer keeps ACT outputs live (defeats DCE)
                    # while removing nearly all DVE streaming work
                    d_out = (accp if c == 0 else tmpp).tile(
                        [128, W], bf16, name="d"
                    )
                    if "noact" not in ablate:
                        nc.scalar.activation(
                            d_out[:, 0:AW], ps[:, 0:AW], AF.Sqrt, **bias_kw,
                        )
                    minv = minvp.tile([128, 1], f32, name="minv")
                    nc.vector.tensor_reduce(
                        minv, d_out[:, 0:4], mybir.AxisListType.X, ALU.min,
                    )
                    nc.sync.dma_start(out=out[b, nt, :], in_=minv)
                    continue
                if c == 0:
                    acc = accp.tile([128, W], bf16, name=f"acc{nt}")
                    accs[nt] = acc
                    if "noact" not in ablate:
                        nc.scalar.activation(
                            acc[:, 0:AW], ps[:, 0:AW], AF.Sqrt, **bias_kw,
                        )
                elif c == 1:
                    tmp = tmpp.tile([128, W], bf16, name="tmp")
                    if "noact" not in ablate:
                        nc.scalar.activation(
                            tmp[:, 0:AW], ps[:, 0:AW], AF.Sqrt, **bias_kw,
                        )
                    if "nodve" not in ablate:
                        ti = b * NT + nt
                        eng = (
                            nc.gpsimd
                            if ("gpsall" in ablate or
                                (ti % 16) in GPS1 and "nogps" not in ablate)
                            else nc.vector
                        )
                        if "fresh1" in ablate:
                            acc2 = accp.tile([128, W], bf16, name=f"acc2_{nt}")
                            eng.tensor_add(
                                acc2[:, 0:AW], accs[nt][:, 0:AW],
                                tmp[:, 0:AW],
                            )
                            accs[nt] = acc2
                        else:
                            eng.tensor_add(
                                accs[nt][:, 0:AW], accs[nt][:, 0:AW],
                                tmp[:, 0:AW],
                            )
                else:
                    tmp = tmpp.tile([128, W], bf16, name="tmp")
                    if "noact" not in ablate:
                        nc.scalar.activation(
                            tmp[:, 0:AW], ps[:, 0:AW], AF.Sqrt, **bias_kw,
                        )
                    if "nodve" not in ablate:
                        ti = b * NT + nt
                        scratch = scrp.tile([128, W], bf16, name="scratch")
                        minv = minvp.tile([128, 1], f32, name="minv")
                        eng = (
                            nc.gpsimd
                            if ("gpsall" in ablate or
                                (ti % 16) in GPS2 and "nogps" not in ablate)
                            else nc.vector
                        )
                        eng.tensor_add(
                            scratch[:, 0:AW], accs[nt][:, 0:AW],
                            tmp[:, 0:AW],
                        )
                        if "nopmin" in ablate or AW != W:
                            nc.vector.tensor_reduce(
                                minv, scratch[:, 0:AW],
                                mybir.AxisListType.X, ALU.min,
                            )
                        else:
                            # 2-level pairwise-min cascade (bf16 2x) then a
                            # short 1x reduce; overlapping middle column is
                            # harmless for min
                            m1 = scrp.tile([128, 993], bf16, name="m1")
                            nc.vector.tensor_tensor(
                                m1, scratch[:, 0:993], scratch[:, 992:1985],
                                ALU.min,
                            )
                            m2 = scrp.tile([128, 497], bf16, name="m2")
                            nc.vector.tensor_tensor(
                                m2, m1[:, 0:497], m1[:, 496:993], ALU.min,
                            )
                            nc.vector.tensor_reduce(
                                minv, m2, mybir.AxisListType.X, ALU.min,
                            )
                        nc.sync.dma_start(out=out[b, nt, :], in_=minv)


def _get_nc():
    if "nc" not in _cache:
        _cache["nc"] = _build_nc()
    return _cache["nc"]


def _prep_inputs(x, shapelets):
    import ml_dtypes

    bf16 = ml_dtypes.bfloat16
    x = np.ascontiguousarray(np.asarray(x), dtype=np.float32)
    sh = np.asarray(shapelets, dtype=np.float32)
    # round shapelets and x to bf16 once; all derived quantities use the
    # rounded values so d2 stays an exact squared distance of the rounded
    # vectors
    shb = sh.astype(bf16).astype(np.float32)
    xb = x.astype(bf16)
    xbf = xb.astype(np.float32)
    xqb = (xbf * xbf).astype(bf16)

    shT = np.transpose(shb, (0, 2, 1))  # (C, S, N)
    wts = np.empty((C, NT, 128, 128), np.float32)
    for nt in range(NT):
        wts[:, nt, :S, :] = -2.0 * shT[:, :, nt * 128 : (nt + 1) * 128]
    wts[:, :, S:, :] = 1.0
    ssq = np.sum(shb * shb, axis=2).reshape(C, NT, 128).astype(np.float32)
    wts_b = np.ascontiguousarray(wts.astype(bf16))
    ssq = np.ascontiguousarray(ssq)
    in_maps = [
        {
            "xs": np.ascontiguousarray(xb[k * BPC : (k + 1) * BPC]),
            "xq": np.ascontiguousarray(xqb[k * BPC : (k + 1) * BPC]),
            "wts": wts_b,
            "ssq": ssq,
        }
        for k in range(NCORES)
    ]
    return in_maps


def _gather(results):
    outs = [np.asarray(r["out"]).reshape(BPC, NSH) for r in results]
    full = np.concatenate(outs, axis=0)  # (64, 256)
    return np.ascontiguousarray(full[:, None, :]).astype(np.float32)


def kernel(x, shapelets):
    from concourse.bass_utils import run_bass_kernel_spmd

    nc = _get_nc()
    in_maps = _prep_inputs(x, shapelets)
    res = run_bass_kernel_spmd(nc, in_maps, core_ids=list(range(NCORES)))
    return _gather(res.results)


def kernel_traced(x, shapelets):
    from concourse.bass_utils import run_bass_kernel_spmd

    nc = _get_nc()
    in_maps = _prep_inputs(x, shapelets)
    res = run_bass_kernel_spmd(nc, in_maps, core_ids=list(range(NCORES)), trace=True)
    return _gather(res.results), res
